# revision 27
# baseline (speedup 1.0000x reference)
"""Trainium2 Bass kernel for nn_CDMTransformer (distance-decay transformer).

Sharding: 8 NeuronCores = 2 batches x 4 head-groups; each core owns one batch
and HC=4 of the 16 heads, plus a 256-token shard for residual/LN.

The execution backend charges a roughly flat per-instruction cost
(DMA ~35us, DVE ~35-60us, PE ~70us, ACT ~130us, XBAR-transpose DMA ~21us)
independent of operand size, so this kernel minimizes instruction count:

  - eltwise decay pipeline batched over stripe-PAIRS x all heads in single
    big-view DVE/ACT ops ([128, 2, HC, S] views, one op per step)
  - suffix sums via one grand tensor_tensor_scan across the whole pair
    (cumulative block totals recovered from the scan's last column;
    per-block Z from a pre-scan row reduce)
  - causal mask + PSUM->SBUF copy fused into one tensor_add with a
    precomputed mask table (stripe-dependent offset APs)
  - log-space distance with |gamma| folded in as 2*ln|g| (exp/ln share one
    ACT table -> no table reloads; back-to-back ACT ops -> fewer semaphores)
  - maxout rescale folded into the fp16 e2 cast (one mul per pair); the
    fp16 e2 buffer aliases the dead raw-score bytes
  - e2/v transposed with XBAR transpose DMAs, one full-width [128, HC*S]
    transpose per stripe (masked/garbage regions are exact zeros, giving
    the zero-padded e2^T that full-width attn@V matmuls need for free)
  - attn@V emitted feature-major (oT = V^T @ e2T) so the out-projection
    consumes it directly; no separate output transpose
  - out-projection partials -> 4-core ReduceScatter; token-sharded LN
  - feature-major regather via element-scatter DMAs + AllGather
"""

import math
from contextlib import ExitStack

import numpy as np

import concourse.bass as bass
import concourse.mybir as mybir
import concourse.tile as tile
from concourse import bacc
from concourse.bass_utils import run_bass_kernel_spmd
from concourse.hw_specs import get_activation_tables as _real_gat


def _gat_one_table(arch):
    # Force every ACT function through natural_log_exp_and_others (this
    # kernel only uses Exp/Ln/Copy/Identity) so the table chooser never
    # alternates sets and inserts reload instructions.
    out = {}
    for name, funcs in _real_gat(arch).items():
        out[name] = funcs if name == "natural_log_exp_and_others" else set()
    return out


try:
    import ml_dtypes

    _BF16 = ml_dtypes.bfloat16
except Exception:  # pragma: no cover
    _BF16 = np.float32

F32 = mybir.dt.float32
F32R = mybir.dt.float32r
BF16 = mybir.dt.bfloat16
F16 = mybir.dt.float16
AF = mybir.ActivationFunctionType
OP = mybir.AluOpType

NEGBIG = -1.0e30
TINY = 1.0e-30


class Cfg:
    def __init__(self, B=2, S=1024, D=1024, H=16, L=4, n_cores=8, repeats=1):
        self.B, self.S, self.D, self.H, self.L = B, S, D, H, L
        self.n_cores = n_cores
        self.repeats = repeats
        self.DH = D // H                    # 64
        self.group = n_cores // B           # 4 cores per batch
        self.HC = H // self.group           # 4 heads per core
        self.HD = self.HC * self.DH         # 256 head-group width
        self.TS = S // self.group           # 256-token shard
        self.NST = S // 128                 # 8 q stripes
        self.FC = D // 128                  # 8 feature chunks
        self.SC = self.TS // 128            # 2 shard chunks
        self.DCC = self.HD // 128           # 2 head-pair blocks
        self.KC = S // 128                  # 8 key blocks

    @property
    def key(self):
        return (self.B, self.S, self.D, self.H, self.L, self.n_cores,
                self.repeats)


def _bc(ap, shape):
    """broadcast_to helper."""
    return ap.broadcast_to(shape)


def build_program(cfg: Cfg):
    saved = bacc.get_activation_tables
    bacc.get_activation_tables = _gat_one_table
    try:
        return _build(cfg)
    finally:
        bacc.get_activation_tables = saved


def _build(c: Cfg):
    nc = bacc.Bacc("TRN2", target_bir_lowering=False, debug=False,
                   num_devices=c.n_cores)
    sc_inv = 1.0 / math.sqrt(c.DH)
    S, HC, L = c.S, c.HC, c.L

    # ---------------- DRAM ----------------
    x0T_d = nc.dram_tensor("x0T", [c.D, S], F32, kind="ExternalInput").ap()
    x0s_d = nc.dram_tensor("x0s", [c.TS, c.D], F32, kind="ExternalInput").ap()
    wqv_d = nc.dram_tensor("wqv", [L, c.D, 2 * c.HD], F32,
                           kind="ExternalInput").ap()
    wo_d = nc.dram_tensor("wo", [L, c.HD, c.D], F32, kind="ExternalInput").ap()
    gneg_d = nc.dram_tensor("gneg", [128, L, HC], F32, kind="ExternalInput").ap()
    lnpos_d = nc.dram_tensor("lnpos", [128, 2 * S], F32, kind="ExternalInput").ap()
    cmask_d = nc.dram_tensor("cmask", [128, S + 128], F32, kind="ExternalInput").ap()
    out_d = nc.dram_tensor("out", [c.TS, c.D], F32, kind="ExternalOutput").ap()

    groups = [[b * c.group + r for r in range(c.group)] for b in range(c.B)]

    apart_d, ared_d, xpiece_d, xall_d = [], [], [], []
    for l in range(L):
        apart_d.append(nc.dram_tensor(f"apart{l}", [S, c.D], F32).ap())
        ared_d.append(nc.dram_tensor(f"ared{l}", [c.TS, c.D], F32).ap())
        if l < L - 1:
            xpiece_d.append(nc.dram_tensor(f"xpiece{l}", [c.D, c.TS], F32).ap())
            xall_d.append(
                nc.dram_tensor(f"xall{l}", [c.group * c.D, c.TS], F32).ap())
        else:
            xpiece_d.append(None)
            xall_d.append(None)

    with tile.TileContext(nc) as tc, ExitStack() as ctx:
        const = ctx.enter_context(tc.tile_pool(name="const", bufs=1))
        persist = ctx.enter_context(tc.tile_pool(name="persist", bufs=1))
        wpool = ctx.enter_context(tc.tile_pool(name="wpool", bufs=1))
        stats = ctx.enter_context(tc.tile_pool(name="stats", bufs=2))
        psS = ctx.enter_context(tc.tile_pool(name="psS", bufs=1, space="PSUM"))
        psV = ctx.enter_context(tc.tile_pool(name="psV", bufs=1, space="PSUM"))

        # ---------------- constants ----------------
        lnpos = const.tile([128, 2 * S], F32)
        nc.sync.dma_start(out=lnpos, in_=lnpos_d)
        cmask = const.tile([128, S + 128], F32)
        nc.sync.dma_start(out=cmask, in_=cmask_d)
        gneg = const.tile([128, L, HC], F32)
        nc.sync.dma_start(out=gneg, in_=gneg_d)
        tiny_c = const.tile([128, 1], F32)
        nc.vector.memset(tiny_c, TINY)
        eps_c = const.tile([128, 1], F32)
        nc.vector.memset(eps_c, 1e-5)

        # ---------------- persistent tiles ----------------
        xt = persist.tile([128, c.FC, S], F32)       # feature-major x
        xs = persist.tile([128, c.SC, c.D], F32)     # token-shard residual
        qt = persist.tile([128, c.DCC, S], F32)      # q/k proj, feature-major
        vT = persist.tile([128, c.DCC, S], F16)     # v proj, feature-major
        vsb = persist.tile([128, c.DCC, c.KC, 128], F16)  # v, token-major
        sbuf = persist.tile([128, 2, HC, S], F32)    # raw scores (pair)
        abuf = persist.tile([128, 2, HC, S], F32)    # decay scratch (pair)
        # e2 (bf16, q-major) aliases sbuf's first half: the raw scores are
        # dead once s2 is formed, and the tile tracker orders the overlap.
        e2q = (sbuf.rearrange("p a h t -> p (a h t)").bitcast(F16)
               [:, :2 * HC * S].rearrange("p (a h t) -> p a h t", a=2, h=HC))
        e2T = persist.tile([128, HC, c.KC, 512], F16)  # e2^T (quad)
        oT = persist.tile([128, c.DCC, S], F32)      # attn out, feature-major

        nc.sync.dma_start(
            out=xt, in_=x0T_d.rearrange("(f p) t -> p f t", p=128))
        nc.sync.dma_start(
            out=xs, in_=x0s_d.rearrange("(s p) d -> p s d", p=128))

        for rep in range(c.repeats):
          for l in range(L):
            # ---------------- weights ----------------
            wqv = wpool.tile([128, c.FC, 2 * c.HD], F32, tag="wqv")
            nc.sync.dma_start(
                out=wqv, in_=wqv_d[l].rearrange("(f p) h -> p f h", p=128))
            wq = wqv[:, :, :c.HD]
            wv = wqv[:, :, c.HD:]
            wo = wpool.tile([128, c.DCC, c.D], F32, tag="wo")
            nc.sync.dma_start(
                out=wo, in_=wo_d[l].rearrange("(e p) d -> p e d", p=128))

            # ---------------- q/k and v projections (feature-major) --------
            # proj[dh128-block dc, tok] = sum_fc W[fc,:,dc].T @ xt[fc, tok];
            # both dc blocks share one [128, 2048] PSUM tile -> one copy each
            for w, dst in ((wq, qt), (wv, vT)):
                ps = psS.tile([128, 2048], F32, tag="ps")
                for dc in range(c.DCC):
                    for half in range(2):
                        o0 = dc * 1024 + half * 512
                        for fc in range(c.FC):
                            nc.tensor.matmul(
                                ps[:, o0:o0 + 512],
                                lhsT=w[:, fc, dc * 128:(dc + 1) * 128],
                                rhs=xt[:, fc, half * 512:(half + 1) * 512],
                                start=(fc == 0), stop=(fc == c.FC - 1))
                nc.vector.tensor_copy(dst.rearrange("p dc t -> p (dc t)"), ps)
            # v -> token-major via one XBAR transpose:
            # vsb[k, dc, kb, dd] = vT[dd, dc, kb*128+k]
            nc.sync.dma_start(
                out=vsb.rearrange("p dc kb d -> p (dc kb) d"),
                in_=vT.rearrange("p dc t -> p (dc t)"), transpose=True)

            # ---------------- attention ----------------
            glh = gneg[:, l, :]  # [128, HC]
            psv_t = []
            for dc in range(c.DCC):
                pvt = psV.tile([128, 1024], F32, tag=f"pv{dc}")
                psv_t.append(pvt)
            for Qb in range(2):
                for pr in range(2 * Qb, 2 * Qb + 2):
                    st0, st1 = 2 * pr, 2 * pr + 1
                    W0, W1 = 128 * (st0 + 1), 128 * (st1 + 1)
                    # raw scores with fused causal mask into sbuf
                    nc.vector.memset(sbuf, NEGBIG)
                    for j, (st, W) in enumerate(((st0, W0), (st1, W1))):
                        qblk = [qt[(h % 2) * 64:(h % 2) * 64 + c.DH, h // 2,
                                   st * 128:(st + 1) * 128] for h in range(HC)]
                        if W <= 512:
                            ps = psS.tile([128, 2048], F32, tag="ps")
                            pv = ps.rearrange("p (h w) -> p h w", h=4)
                            for h in range(HC):
                                nc.tensor.matmul(
                                    pv[:, h, :W], lhsT=qblk[h],
                                    rhs=qt[(h % 2) * 64:(h % 2) * 64 + c.DH,
                                           h // 2, :W],
                                    start=True, stop=True)
                            mrow = bass.AP(
                                tensor=cmask.tensor,
                                offset=cmask.offset + S - st * 128,
                                ap=[list(cmask.ap[0]), [0, HC], [1, W]])
                            nc.vector.tensor_add(
                                sbuf[:, j, :, :W], pv[:, :, :W], mrow)
                        else:
                            for hp in range(2):
                                ps = psS.tile([128, 2048], F32, tag="ps")
                                pv = ps.rearrange("p (h w) -> p h w", h=2)
                                for hh in range(2):
                                    h = hp * 2 + hh
                                    for nb in range((W + 511) // 512):
                                        n0, n1 = nb * 512, min(W, nb * 512 + 512)
                                        nc.tensor.matmul(
                                            pv[:, hh, n0:n1], lhsT=qblk[h],
                                            rhs=qt[(h % 2) * 64:(h % 2) * 64 + c.DH,
                                                   h // 2, n0:n1],
                                            start=True, stop=True)
                                mrow = bass.AP(
                                    tensor=cmask.tensor,
                                    offset=cmask.offset + S - st * 128,
                                    ap=[list(cmask.ap[0]), [0, 2], [1, W]])
                                nc.vector.tensor_add(
                                    sbuf[:, j, hp * 2:hp * 2 + 2, :W],
                                    pv[:, :, :W], mrow)

                    # -------- batched decay pipeline on [128, 2, HC, S] -----
                    sflat = sbuf.rearrange("p a h t -> p (a h t)")
                    aflat = abuf.rearrange("p a h t -> p (a h t)")
                    # e = exp(s/sqrt(dh));   (masked/garbage -> 0)
                    nc.scalar.activation(out=aflat, in_=sflat, func=AF.Exp,
                                         scale=sc_inv)
                    # per-block row sums Z (pre-scan) for the 1/Z term
                    zrow = stats.tile([128, 2, HC], F32, tag="z")
                    nc.vector.tensor_reduce(out=zrow, in_=abuf,
                                            axis=mybir.AxisListType.X, op=OP.add)
                    # grand prefix scan across the whole pair
                    nc.vector.tensor_tensor_scan(
                        out=aflat, data0=aflat, data1=aflat,
                        initial=0.0, op0=OP.add, op1=OP.bypass)
                    # cumulative-through-block totals (for the suffix subtract)
                    ctot = stats.tile([128, 2, HC], F32, tag="c")
                    nc.vector.tensor_copy(ctot.unsqueeze(3),
                                          abuf[:, :, :, S - 1:S])
                    # lnzg = ln(Z) - ln(g^2)   (per stripe,head)
                    lnz = stats.tile([128, 2, HC], F32, tag="lnz")
                    nc.scalar.activation(out=lnz, in_=zrow, func=AF.Ln,
                                         bias=tiny_c)
                    lnzg = stats.tile([128, 2, HC], F32, tag="lnzg")
                    nc.vector.tensor_sub(
                        lnzg, lnz, _bc(glh.unsqueeze(1), (128, 2, HC)))
                    # sm = min(pref - Z, 0) = -(strict suffix)
                    nc.vector.tensor_sub(abuf, abuf, _bc(ctot.unsqueeze(3),
                                                         (128, 2, HC, S)))
                    nc.vector.tensor_scalar(out=aflat, in0=aflat, scalar1=0.0,
                                            scalar2=None, op0=OP.min)
                    # Ldist = ln(suffix) + ln(pos) - lnzg;
                    # |g|*dist = exp(0.5*Ldist); eff = exp(-|g|*dist)
                    nc.scalar.activation(out=aflat, in_=aflat, func=AF.Ln,
                                         scale=-1.0, bias=tiny_c)
                    lnp0 = bass.AP(
                        tensor=lnpos.tensor,
                        offset=lnpos.offset + S - st0 * 128,
                        ap=[list(lnpos.ap[0]), [-128, 2], [0, HC], [1, S]])
                    nc.vector.tensor_add(abuf, abuf, lnp0)
                    nc.vector.tensor_sub(abuf, abuf, _bc(lnzg.unsqueeze(3),
                                                         (128, 2, HC, S)))
                    nc.scalar.activation(out=aflat, in_=aflat, func=AF.Exp,
                                         scale=0.5)
                    nc.scalar.activation(out=aflat, in_=aflat, func=AF.Exp,
                                         scale=-1.0)
                    # s2 = (s/sqrt(dh)) * eff;  e2 = exp(s2)  (bf16)
                    nc.vector.scalar_tensor_tensor(
                        out=aflat, in0=sflat, scalar=sc_inv, in1=aflat,
                        op0=OP.mult, op1=OP.mult)
                    e2f = e2q.rearrange("p a h t -> p (a h t)")
                    nc.scalar.activation(out=e2f, in_=aflat, func=AF.Exp)
                    # maxout: t2 = 1/max(m2, Z2/5); fold into e2
                    z2 = stats.tile([128, 2, HC], F32, tag="z2")
                    nc.vector.tensor_reduce(out=z2, in_=e2q,
                                            axis=mybir.AxisListType.X, op=OP.add)
                    m2 = stats.tile([128, 2, HC], F32, tag="m2")
                    nc.vector.tensor_reduce(out=m2, in_=e2q,
                                            axis=mybir.AxisListType.X, op=OP.max)
                    vmx = stats.tile([128, 2, HC], F32, tag="vm")
                    nc.vector.scalar_tensor_tensor(
                        out=vmx, in0=z2, scalar=0.2, in1=m2,
                        op0=OP.mult, op1=OP.max)
                    nc.vector.tensor_scalar_add(vmx, vmx, TINY)
                    t2 = stats.tile([128, 2, HC], F32, tag="t2")
                    nc.vector.reciprocal(t2, vmx)
                    nc.vector.tensor_mul(e2q, e2q, _bc(t2.unsqueeze(3),
                                                       (128, 2, HC, S)))
                    # transpose e2 into the quad buffer: one full-width XBAR
                    # per stripe (garbage/invalid-kb regions are exact zeros,
                    # which is what the full-width attn@V matmuls need)
                    for j, st in enumerate((st0, st1)):
                        q0 = (st % 4) * 128
                        nc.sync.dma_start(
                            out=e2T[:, :, :, q0:q0 + 128].rearrange(
                                "p h kb q -> p (h kb) q"),
                            in_=e2q[:, j].rearrange("p h t -> p (h t)"),
                            transpose=True)

                # -------- attn@V for this quad: oT = V^T @ e2T --------------
                nkb = 4 * Qb + 4
                for dc in range(c.DCC):
                    ps = psv_t[dc]
                    for hh in range(2):
                        h = 2 * dc + hh
                        for kb in range(nkb):
                            nc.tensor.matmul(
                                ps[hh * 64:(hh + 1) * 64,
                                   Qb * 512:(Qb + 1) * 512],
                                lhsT=vsb[:, dc, kb, hh * 64:(hh + 1) * 64],
                                rhs=e2T[:, h, kb, :],
                                start=(kb == 0), stop=(kb == nkb - 1))
            for dc in range(c.DCC):
                nc.vector.tensor_copy(oT[:, dc, :], psv_t[dc])

            # ---------------- out-projection partials -------------------
            apsb = sbuf.rearrange("p a h t -> p (a h) t")  # reuse as [128,8,1024]
            for tbp in range(c.KC // 2):
                ps = psS.tile([128, 2048], F32, tag="ps")
                for j2 in range(2):
                    tb = 2 * tbp + j2
                    for half in range(2):
                        o0 = j2 * 1024 + half * 512
                        for dc in range(c.DCC):
                            nc.tensor.matmul(
                                ps[:, o0:o0 + 512],
                                lhsT=oT[:, dc, tb * 128:(tb + 1) * 128],
                                rhs=wo[:, dc, half * 512:(half + 1) * 512],
                                start=(dc == 0), stop=(dc == c.DCC - 1))
                nc.vector.tensor_copy(apsb[:, 2 * tbp:2 * tbp + 2, :], ps)
            nc.sync.dma_start(
                out=apart_d[l].rearrange("(t p) d -> p t d", p=128), in_=apsb)
            nc.gpsimd.collective_compute(
                "ReduceScatter", OP.add, replica_groups=groups,
                ins=[apart_d[l]], outs=[ared_d[l]])
            ar = abuf.rearrange("p a h t -> p (a h) t")  # reuse [128,8,1024]
            nc.sync.dma_start(
                out=ar[:, :c.SC, :],
                in_=ared_d[l].rearrange("(s p) d -> p s d", p=128))

            # ---------------- residual + layernorm ----------------------
            xa = ar[:, c.SC:2 * c.SC, :]
            nc.vector.tensor_add(xa, xs, ar[:, :c.SC, :])
            mean = stats.tile([128, c.SC], F32, tag="mu")
            nc.vector.tensor_reduce(out=mean, in_=xa,
                                    axis=mybir.AxisListType.X, op=OP.add)
            nc.vector.tensor_scalar_mul(mean, mean, -1.0 / c.D)
            nc.vector.tensor_add(xa, xa, _bc(mean.unsqueeze(2),
                                             (128, c.SC, c.D)))
            sq = ar[:, 2 * c.SC:3 * c.SC, :]
            nc.vector.tensor_mul(sq, xa, xa)
            var = stats.tile([128, c.SC], F32, tag="var")
            nc.vector.tensor_reduce(out=var, in_=sq,
                                    axis=mybir.AxisListType.X, op=OP.add)
            lnv = stats.tile([128, c.SC], F32, tag="lnv")
            nc.scalar.activation(out=lnv, in_=var, func=AF.Ln, scale=1.0 / c.D,
                                 bias=eps_c)
            rstd = stats.tile([128, c.SC], F32, tag="rstd")
            nc.scalar.activation(out=rstd, in_=lnv, func=AF.Exp, scale=-0.5)
            last = (rep == c.repeats - 1) and (l == L - 1)
            nc.vector.tensor_mul(xs, xa, _bc(rstd.unsqueeze(2),
                                             (128, c.SC, c.D)))

            if not last:
                # scatter-write shard feature-major, AllGather, reload xt
                lx = l if l < L - 1 else 0
                for sc in range(c.SC):
                    dst = bass.AP(
                        tensor=xpiece_d[lx].tensor, offset=sc * 128,
                        ap=[[1, 128], [c.TS, c.D]])
                    with nc.allow_non_contiguous_dma(reason="transpose"):
                        nc.sync.dma_start(out=dst, in_=xs[:, sc, :])
                nc.gpsimd.collective_compute(
                    "AllGather", OP.bypass, replica_groups=groups,
                    ins=[xpiece_d[lx]], outs=[xall_d[lx]])
                for r in range(c.group):
                    nc.sync.dma_start(
                        out=xt[:, :, r * c.TS:(r + 1) * c.TS],
                        in_=xall_d[lx][r * c.D:(r + 1) * c.D, :].rearrange(
                            "(f p) t -> p f t", p=128))
            else:
                # final layernorm on the shard -> output
                xf = xs
                mean2 = stats.tile([128, c.SC], F32, tag="mu2")
                nc.vector.tensor_reduce(out=mean2, in_=xf,
                                        axis=mybir.AxisListType.X, op=OP.add)
                nc.vector.tensor_scalar_mul(mean2, mean2, -1.0 / c.D)
                nc.vector.tensor_add(xf, xf, _bc(mean2.unsqueeze(2),
                                                 (128, c.SC, c.D)))
                sq2 = ar[:, :c.SC, :]
                nc.vector.tensor_mul(sq2, xf, xf)
                var2 = stats.tile([128, c.SC], F32, tag="var2")
                nc.vector.tensor_reduce(out=var2, in_=sq2,
                                        axis=mybir.AxisListType.X, op=OP.add)
                lnv2 = stats.tile([128, c.SC], F32, tag="lnv2")
                nc.scalar.activation(out=lnv2, in_=var2, func=AF.Ln,
                                     scale=1.0 / c.D, bias=eps_c)
                rstd2 = stats.tile([128, c.SC], F32, tag="rstd2")
                nc.scalar.activation(out=rstd2, in_=lnv2, func=AF.Exp,
                                     scale=-0.5)
                fo = ar[:, c.SC:2 * c.SC, :]
                nc.vector.tensor_mul(fo, xf, _bc(rstd2.unsqueeze(2),
                                                 (128, c.SC, c.D)))
                nc.sync.dma_start(
                    out=out_d.rearrange("(s p) d -> p s d", p=128),
                    in_=fo)

    nc.compile()
    return nc


# ---------------------------------------------------------------------------
# host side
# ---------------------------------------------------------------------------

def make_in_maps(cfg: Cfg, q, Wq, Wv, Wo, gammas):
    c = cfg
    q = np.asarray(q, np.float32)
    Wq = np.asarray(Wq, np.float32)
    Wv = np.asarray(Wv, np.float32)
    Wo = np.asarray(Wo, np.float32)
    gammas = np.asarray(gammas, np.float32)
    S = c.S

    p = np.arange(128)[:, None]
    # lnpos[p, c] = ln(|p + S - c|), read at c = k + S - st*128
    cc = np.arange(2 * S)[None, :]
    posv = np.abs(p + S - cc).astype(np.float32)
    with np.errstate(divide="ignore"):
        lnpos = np.where(posv > 0, np.log(posv), NEGBIG).astype(np.float32)
    # cmask[p, c'] = 0 if (c' - S) < p else NEGBIG, read at c' = k + S - st*128
    cp = np.arange(S + 128)[None, :]
    cmask = np.where((cp - S) < p, 0.0, NEGBIG).astype(np.float32)

    in_maps = []
    for core in range(c.n_cores):
        b, hg = divmod(core, c.group)
        h0 = hg * c.HC
        cols = slice(h0 * c.DH, (h0 + c.HC) * c.DH)
        # 2*ln|gamma| so that exp(0.5*(L - lnZ + ln g^2)) = |g|*dist
        gn = 2.0 * np.log(np.maximum(np.abs(gammas[:, h0:h0 + c.HC]), 1e-20))
        in_maps.append({
            "x0T": np.ascontiguousarray(q[b].T),
            "x0s": np.ascontiguousarray(q[b][hg * c.TS:(hg + 1) * c.TS]),
            "wqv": np.ascontiguousarray(
                np.concatenate([Wq[:, :, cols], Wv[:, :, cols]], axis=2)),
            "wo": np.ascontiguousarray(Wo[:, cols, :]),
            "gneg": np.broadcast_to(gn[None], (128, c.L, c.HC)).copy(),
            "lnpos": lnpos,
            "cmask": cmask,
        })
    return in_maps


def assemble_out(cfg: Cfg, results):
    c = cfg
    out = np.empty((c.B, c.S, c.D), np.float32)
    for core in range(c.n_cores):
        b, hg = divmod(core, c.group)
        out[b, hg * c.TS:(hg + 1) * c.TS] = results[core]["out"]
    return out


_PROGRAM_CACHE = {}


def get_program(cfg: Cfg):
    nc = _PROGRAM_CACHE.get(cfg.key)
    if nc is None:
        nc = build_program(cfg)
        _PROGRAM_CACHE[cfg.key] = nc
    return nc


def kernel(**inputs):
    cfg = Cfg()
    nc = get_program(cfg)
    in_maps = make_in_maps(
        cfg, inputs["q"], inputs["Wq"], inputs["Wv"], inputs["Wo"],
        inputs["gammas"])
    res = run_bass_kernel_spmd(nc, in_maps, list(range(cfg.n_cores)))
    return assemble_out(cfg, res.results)


# revision 28
# speedup vs baseline: 1.1221x; 1.1221x over previous
"""Trainium2 Bass kernel for nn_CDMTransformer (distance-decay transformer).

Sharding: 8 NeuronCores = 2 batches x 4 head-groups; each core owns one batch
and HC=4 of the 16 heads, plus a 256-token shard for residual/LN.

The execution backend charges a roughly flat per-instruction cost
(DMA ~35us, DVE ~35-60us, PE ~70us, ACT ~130us, XBAR-transpose DMA ~21us)
independent of operand size, so this kernel minimizes instruction count:

  - eltwise decay pipeline batched over stripe-PAIRS x all heads in single
    big-view DVE/ACT ops ([128, 2, HC, S] views, one op per step)
  - suffix sums via one grand tensor_tensor_scan across the whole pair
    (cumulative block totals recovered from the scan's last column;
    per-block Z from a pre-scan row reduce)
  - causal mask + PSUM->SBUF copy fused into one tensor_add with a
    precomputed mask table (stripe-dependent offset APs)
  - log-space distance with |gamma| folded in as 2*ln|g| (exp/ln share one
    ACT table -> no table reloads; back-to-back ACT ops -> fewer semaphores)
  - maxout rescale folded into the fp16 e2 cast (one mul per pair); the
    fp16 e2 buffer aliases the dead raw-score bytes
  - e2/v transposed with XBAR transpose DMAs, one full-width [128, HC*S]
    transpose per stripe (masked/garbage regions are exact zeros, giving
    the zero-padded e2^T that full-width attn@V matmuls need for free)
  - attn@V emitted feature-major (oT = V^T @ e2T) so the out-projection
    consumes it directly; no separate output transpose
  - out-projection partials -> 4-core ReduceScatter; token-sharded LN
  - feature-major regather via element-scatter DMAs + AllGather
"""

import math
from contextlib import ExitStack

import numpy as np

import concourse.bass as bass
import concourse.mybir as mybir
import concourse.tile as tile
from concourse import bacc
from concourse.bass_utils import run_bass_kernel_spmd
from concourse.hw_specs import get_activation_tables as _real_gat


def _gat_one_table(arch):
    # Force every ACT function through natural_log_exp_and_others (this
    # kernel only uses Exp/Ln/Copy/Identity) so the table chooser never
    # alternates sets and inserts reload instructions.
    out = {}
    for name, funcs in _real_gat(arch).items():
        out[name] = funcs if name == "natural_log_exp_and_others" else set()
    return out


try:
    import ml_dtypes

    _BF16 = ml_dtypes.bfloat16
except Exception:  # pragma: no cover
    _BF16 = np.float32

F32 = mybir.dt.float32
F32R = mybir.dt.float32r
BF16 = mybir.dt.bfloat16
F16 = mybir.dt.float16
AF = mybir.ActivationFunctionType
OP = mybir.AluOpType

NEGBIG = -1.0e30
TINY = 1.0e-30


class Cfg:
    def __init__(self, B=2, S=1024, D=1024, H=16, L=4, n_cores=8, repeats=1):
        self.B, self.S, self.D, self.H, self.L = B, S, D, H, L
        self.n_cores = n_cores
        self.repeats = repeats
        self.DH = D // H                    # 64
        self.group = n_cores // B           # 4 cores per batch
        self.HC = H // self.group           # 4 heads per core
        self.HD = self.HC * self.DH         # 256 head-group width
        self.TS = S // self.group           # 256-token shard
        self.NST = S // 128                 # 8 q stripes
        self.FC = D // 128                  # 8 feature chunks
        self.SC = self.TS // 128            # 2 shard chunks
        self.DCC = self.HD // 128           # 2 head-pair blocks
        self.KC = S // 128                  # 8 key blocks

    @property
    def key(self):
        return (self.B, self.S, self.D, self.H, self.L, self.n_cores,
                self.repeats)


def _bc(ap, shape):
    """broadcast_to helper."""
    return ap.broadcast_to(shape)


def build_program(cfg: Cfg):
    saved = bacc.get_activation_tables
    bacc.get_activation_tables = _gat_one_table
    try:
        return _build(cfg)
    finally:
        bacc.get_activation_tables = saved


def _build(c: Cfg):
    nc = bacc.Bacc("TRN2", target_bir_lowering=False, debug=False,
                   num_devices=c.n_cores)
    sc_inv = 1.0 / math.sqrt(c.DH)
    S, HC, L = c.S, c.HC, c.L

    # ---------------- DRAM ----------------
    x0T_d = nc.dram_tensor("x0T", [c.D, S], F32, kind="ExternalInput").ap()
    x0s_d = nc.dram_tensor("x0s", [c.TS, c.D], F32, kind="ExternalInput").ap()
    wqv_d = nc.dram_tensor("wqv", [L, c.D, 2 * c.HD], F32,
                           kind="ExternalInput").ap()
    wo_d = nc.dram_tensor("wo", [L, c.HD, c.D], F32, kind="ExternalInput").ap()
    gneg_d = nc.dram_tensor("gneg", [128, L, HC], F32, kind="ExternalInput").ap()
    lnpos_d = nc.dram_tensor("lnpos", [128, 2 * S], F32, kind="ExternalInput").ap()
    cmask_d = nc.dram_tensor("cmask", [128, S + 128], F32, kind="ExternalInput").ap()
    out_d = nc.dram_tensor("out", [c.TS, c.D], F32, kind="ExternalOutput").ap()

    groups = [[b * c.group + r for r in range(c.group)] for b in range(c.B)]

    apart_d, ared_d, xpiece_d, xall_d = [], [], [], []
    for l in range(L):
        apart_d.append(nc.dram_tensor(f"apart{l}", [S, c.D], F32).ap())
        ared_d.append(nc.dram_tensor(f"ared{l}", [c.TS, c.D], F32).ap())
        if l < L - 1:
            xpiece_d.append(nc.dram_tensor(f"xpiece{l}", [c.D, c.TS], F32).ap())
            xall_d.append(
                nc.dram_tensor(f"xall{l}", [c.group * c.D, c.TS], F32).ap())
        else:
            xpiece_d.append(None)
            xall_d.append(None)

    with tile.TileContext(nc) as tc, ExitStack() as ctx:
        const = ctx.enter_context(tc.tile_pool(name="const", bufs=1))
        persist = ctx.enter_context(tc.tile_pool(name="persist", bufs=1))
        wpool = ctx.enter_context(tc.tile_pool(name="wpool", bufs=1))
        stats = ctx.enter_context(tc.tile_pool(name="stats", bufs=2))
        psS = ctx.enter_context(tc.tile_pool(name="psS", bufs=1, space="PSUM"))
        psV = ctx.enter_context(tc.tile_pool(name="psV", bufs=1, space="PSUM"))

        # ---------------- constants ----------------
        lnpos = const.tile([128, 2 * S], F32)
        nc.sync.dma_start(out=lnpos, in_=lnpos_d)
        cmask = const.tile([128, S + 128], F32)
        nc.sync.dma_start(out=cmask, in_=cmask_d)
        gneg = const.tile([128, L, HC], F32)
        nc.sync.dma_start(out=gneg, in_=gneg_d)
        tiny_c = const.tile([128, 1], F32)
        nc.vector.memset(tiny_c, TINY)
        eps_c = const.tile([128, 1], F32)
        nc.vector.memset(eps_c, 1e-5)

        # ---------------- persistent tiles ----------------
        xt = persist.tile([128, c.FC, S], F32)       # feature-major x
        xs = persist.tile([128, c.SC, c.D], F32)     # token-shard residual
        qt = persist.tile([128, c.DCC, S], F32)      # q/k proj, feature-major
        vT = persist.tile([128, c.DCC, S], F16)     # v proj, feature-major
        vsb = persist.tile([128, c.DCC, c.KC, 128], F16)  # v, token-major
        sbuf = persist.tile([128, 2, HC, S], F32)    # raw scores (pair)
        abuf = persist.tile([128, 2, HC, S], F32)    # decay scratch (pair)
        # e2 (bf16, q-major) aliases sbuf's first half: the raw scores are
        # dead once s2 is formed, and the tile tracker orders the overlap.
        e2q = (sbuf.rearrange("p a h t -> p (a h t)").bitcast(F16)
               [:, :2 * HC * S].rearrange("p (a h t) -> p a h t", a=2, h=HC))
        e2T = persist.tile([128, HC, c.KC, 512], F16)  # e2^T (quad)
        oT = persist.tile([128, c.DCC, S], F32)      # attn out, feature-major

        nc.sync.dma_start(
            out=xt, in_=x0T_d.rearrange("(f p) t -> p f t", p=128))
        nc.sync.dma_start(
            out=xs, in_=x0s_d.rearrange("(s p) d -> p s d", p=128))

        for rep in range(c.repeats):
          for l in range(L):
            # ---------------- weights ----------------
            wqv = wpool.tile([128, c.FC, 2 * c.HD], F32, tag="wqv")
            nc.sync.dma_start(
                out=wqv, in_=wqv_d[l].rearrange("(f p) h -> p f h", p=128))
            wq = wqv[:, :, :c.HD]
            wv = wqv[:, :, c.HD:]
            wo = wpool.tile([128, c.DCC, c.D], F32, tag="wo")
            nc.sync.dma_start(
                out=wo, in_=wo_d[l].rearrange("(e p) d -> p e d", p=128))

            # ---------------- q/k and v projections (feature-major) --------
            # proj[dh128-block dc, tok] = sum_fc W[fc,:,dc].T @ xt[fc, tok];
            # both dc blocks share one [128, 2048] PSUM tile -> one copy each
            for w, dst in ((wq, qt), (wv, vT)):
                ps = psS.tile([128, 2048], F32, tag="ps")
                for dc in range(c.DCC):
                    for half in range(2):
                        o0 = dc * 1024 + half * 512
                        for fc in range(c.FC):
                            nc.tensor.matmul(
                                ps[:, o0:o0 + 512],
                                lhsT=w[:, fc, dc * 128:(dc + 1) * 128],
                                rhs=xt[:, fc, half * 512:(half + 1) * 512],
                                start=(fc == 0), stop=(fc == c.FC - 1))
                nc.vector.tensor_copy(dst.rearrange("p dc t -> p (dc t)"), ps)
            # v -> token-major via one XBAR transpose:
            # vsb[k, dc, kb, dd] = vT[dd, dc, kb*128+k]
            nc.sync.dma_start(
                out=vsb.rearrange("p dc kb d -> p (dc kb) d"),
                in_=vT.rearrange("p dc t -> p (dc t)"), transpose=True)

            # ---------------- attention ----------------
            glh = gneg[:, l, :]  # [128, HC]
            psv_t = []
            for dc in range(c.DCC):
                pvt = psV.tile([128, 1024], F32, tag=f"pv{dc}")
                psv_t.append(pvt)
            for Qb in range(2):
                for pr in range(2 * Qb, 2 * Qb + 2):
                    st0, st1 = 2 * pr, 2 * pr + 1
                    W0, W1 = 128 * (st0 + 1), 128 * (st1 + 1)
                    # raw scores with fused causal mask into sbuf
                    nc.vector.memset(sbuf, NEGBIG)
                    for j, (st, W) in enumerate(((st0, W0), (st1, W1))):
                        qblk = [qt[(h % 2) * 64:(h % 2) * 64 + c.DH, h // 2,
                                   st * 128:(st + 1) * 128] for h in range(HC)]
                        if W <= 512:
                            ps = psS.tile([128, 2048], F32, tag="ps")
                            pv = ps.rearrange("p (h w) -> p h w", h=4)
                            for h in range(HC):
                                nc.tensor.matmul(
                                    pv[:, h, :W], lhsT=qblk[h],
                                    rhs=qt[(h % 2) * 64:(h % 2) * 64 + c.DH,
                                           h // 2, :W],
                                    start=True, stop=True)
                            mrow = bass.AP(
                                tensor=cmask.tensor,
                                offset=cmask.offset + S - st * 128,
                                ap=[list(cmask.ap[0]), [0, HC], [1, W]])
                            nc.vector.tensor_add(
                                sbuf[:, j, :, :W], pv[:, :, :W], mrow)
                        else:
                            for hp in range(2):
                                ps = psS.tile([128, 2048], F32, tag="ps")
                                pv = ps.rearrange("p (h w) -> p h w", h=2)
                                for hh in range(2):
                                    h = hp * 2 + hh
                                    for nb in range((W + 511) // 512):
                                        n0, n1 = nb * 512, min(W, nb * 512 + 512)
                                        nc.tensor.matmul(
                                            pv[:, hh, n0:n1], lhsT=qblk[h],
                                            rhs=qt[(h % 2) * 64:(h % 2) * 64 + c.DH,
                                                   h // 2, n0:n1],
                                            start=True, stop=True)
                                mrow = bass.AP(
                                    tensor=cmask.tensor,
                                    offset=cmask.offset + S - st * 128,
                                    ap=[list(cmask.ap[0]), [0, 2], [1, W]])
                                nc.vector.tensor_add(
                                    sbuf[:, j, hp * 2:hp * 2 + 2, :W],
                                    pv[:, :, :W], mrow)

                    # -------- batched decay pipeline on [128, 2, HC, S] -----
                    sflat = sbuf.rearrange("p a h t -> p (a h t)")
                    aflat = abuf.rearrange("p a h t -> p (a h t)")
                    # e = exp(s/sqrt(dh));   (masked/garbage -> 0)
                    nc.scalar.activation(out=aflat, in_=sflat, func=AF.Exp,
                                         scale=sc_inv)
                    # per-block row sums Z (pre-scan) for the 1/Z term
                    zrow = stats.tile([128, 2, HC], F32, tag="z")
                    nc.vector.tensor_reduce(out=zrow, in_=abuf,
                                            axis=mybir.AxisListType.X, op=OP.add)
                    # grand prefix scan across the whole pair
                    nc.vector.tensor_tensor_scan(
                        out=aflat, data0=aflat, data1=aflat,
                        initial=0.0, op0=OP.add, op1=OP.bypass)
                    # cumulative-through-block totals (for the suffix subtract)
                    ctot = stats.tile([128, 2, HC], F32, tag="c")
                    nc.vector.tensor_copy(ctot.unsqueeze(3),
                                          abuf[:, :, :, S - 1:S])
                    # lnzg = ln(Z) - ln(g^2)   (per stripe,head)
                    lnz = stats.tile([128, 2, HC], F32, tag="lnz")
                    nc.scalar.activation(out=lnz, in_=zrow, func=AF.Ln,
                                         bias=tiny_c)
                    lnzg = stats.tile([128, 2, HC], F32, tag="lnzg")
                    nc.vector.tensor_sub(
                        lnzg, lnz, _bc(glh.unsqueeze(1), (128, 2, HC)))
                    # sm = min(pref - Z, 0) = -(strict suffix)
                    nc.vector.tensor_sub(abuf, abuf, _bc(ctot.unsqueeze(3),
                                                         (128, 2, HC, S)))
                    nc.vector.tensor_scalar(out=aflat, in0=aflat, scalar1=0.0,
                                            scalar2=None, op0=OP.min)
                    # Ldist = ln(suffix) + ln(pos) - lnzg;
                    # |g|*dist = exp(0.5*Ldist); eff = exp(-|g|*dist)
                    nc.scalar.activation(out=aflat, in_=aflat, func=AF.Ln,
                                         scale=-1.0, bias=tiny_c)
                    lnp0 = bass.AP(
                        tensor=lnpos.tensor,
                        offset=lnpos.offset + S - st0 * 128,
                        ap=[list(lnpos.ap[0]), [-128, 2], [0, HC], [1, S]])
                    nc.vector.tensor_add(abuf, abuf, lnp0)
                    nc.vector.tensor_sub(abuf, abuf, _bc(lnzg.unsqueeze(3),
                                                         (128, 2, HC, S)))
                    nc.scalar.activation(out=aflat, in_=aflat, func=AF.Exp,
                                         scale=0.5)
                    nc.scalar.activation(out=aflat, in_=aflat, func=AF.Exp,
                                         scale=-1.0)
                    # s2 = (s/sqrt(dh)) * eff;  e2 = exp(s2)  (bf16)
                    nc.vector.scalar_tensor_tensor(
                        out=aflat, in0=sflat, scalar=sc_inv, in1=aflat,
                        op0=OP.mult, op1=OP.mult)
                    e2f = e2q.rearrange("p a h t -> p (a h t)")
                    nc.scalar.activation(out=e2f, in_=aflat, func=AF.Exp)
                    # maxout: t2 = 1/max(m2, Z2/5); fold into e2
                    z2 = stats.tile([128, 2, HC], F32, tag="z2")
                    nc.vector.tensor_reduce(out=z2, in_=e2q,
                                            axis=mybir.AxisListType.X, op=OP.add)
                    m2 = stats.tile([128, 2, HC], F32, tag="m2")
                    nc.vector.tensor_reduce(out=m2, in_=e2q,
                                            axis=mybir.AxisListType.X, op=OP.max)
                    vmx = stats.tile([128, 2, HC], F32, tag="vm")
                    nc.vector.scalar_tensor_tensor(
                        out=vmx, in0=z2, scalar=0.2, in1=m2,
                        op0=OP.mult, op1=OP.max)
                    nc.vector.tensor_scalar_add(vmx, vmx, TINY)
                    t2 = stats.tile([128, 2, HC], F32, tag="t2")
                    nc.vector.reciprocal(t2, vmx)
                    nc.vector.tensor_mul(e2q, e2q, _bc(t2.unsqueeze(3),
                                                       (128, 2, HC, S)))
                    # transpose e2 into the quad buffer: one full-width XBAR
                    # per stripe (garbage/invalid-kb regions are exact zeros,
                    # which is what the full-width attn@V matmuls need)
                    for j, st in enumerate((st0, st1)):
                        q0 = (st % 4) * 128
                        nc.sync.dma_start(
                            out=e2T[:, :, :, q0:q0 + 128].rearrange(
                                "p h kb q -> p (h kb) q"),
                            in_=e2q[:, j].rearrange("p h t -> p (h t)"),
                            transpose=True)

                # -------- attn@V for this quad: oT = V^T @ e2T --------------
                nkb = 4 * Qb + 4
                for dc in range(c.DCC):
                    ps = psv_t[dc]
                    for hh in range(2):
                        h = 2 * dc + hh
                        for kb in range(nkb):
                            nc.tensor.matmul(
                                ps[hh * 64:(hh + 1) * 64,
                                   Qb * 512:(Qb + 1) * 512],
                                lhsT=vsb[:, dc, kb, hh * 64:(hh + 1) * 64],
                                rhs=e2T[:, h, kb, :],
                                start=(kb == 0), stop=(kb == nkb - 1))
            for dc in range(c.DCC):
                nc.vector.tensor_copy(oT[:, dc, :], psv_t[dc])

            # ---------------- out-projection partials -------------------
            apsb = sbuf.rearrange("p a h t -> p (a h) t")  # reuse as [128,8,1024]
            for tbp in range(c.KC // 2):
                ps = psS.tile([128, 2048], F32, tag="ps")
                for j2 in range(2):
                    tb = 2 * tbp + j2
                    for half in range(2):
                        o0 = j2 * 1024 + half * 512
                        for dc in range(c.DCC):
                            nc.tensor.matmul(
                                ps[:, o0:o0 + 512],
                                lhsT=oT[:, dc, tb * 128:(tb + 1) * 128],
                                rhs=wo[:, dc, half * 512:(half + 1) * 512],
                                start=(dc == 0), stop=(dc == c.DCC - 1))
                nc.vector.tensor_copy(apsb[:, 2 * tbp:2 * tbp + 2, :], ps)
            nc.sync.dma_start(
                out=apart_d[l].rearrange("(t p) d -> p t d", p=128), in_=apsb)
            nc.gpsimd.collective_compute(
                "ReduceScatter", OP.add, replica_groups=groups,
                ins=[apart_d[l]], outs=[ared_d[l]])
            ar = abuf.rearrange("p a h t -> p (a h) t")  # reuse [128,8,1024]
            nc.sync.dma_start(
                out=ar[:, :c.SC, :],
                in_=ared_d[l].rearrange("(s p) d -> p s d", p=128))

            # ---------------- residual + layernorm ----------------------
            xa = ar[:, c.SC:2 * c.SC, :]
            nc.vector.tensor_add(xa, xs, ar[:, :c.SC, :])
            mean = stats.tile([128, c.SC], F32, tag="mu")
            nc.vector.tensor_reduce(out=mean, in_=xa,
                                    axis=mybir.AxisListType.X, op=OP.add)
            nc.vector.tensor_scalar_mul(mean, mean, -1.0 / c.D)
            nc.vector.tensor_add(xa, xa, _bc(mean.unsqueeze(2),
                                             (128, c.SC, c.D)))
            sq = ar[:, 2 * c.SC:3 * c.SC, :]
            nc.vector.tensor_mul(sq, xa, xa)
            var = stats.tile([128, c.SC], F32, tag="var")
            nc.vector.tensor_reduce(out=var, in_=sq,
                                    axis=mybir.AxisListType.X, op=OP.add)
            lnv = stats.tile([128, c.SC], F32, tag="lnv")
            nc.scalar.activation(out=lnv, in_=var, func=AF.Ln, scale=1.0 / c.D,
                                 bias=eps_c)
            rstd = stats.tile([128, c.SC], F32, tag="rstd")
            nc.scalar.activation(out=rstd, in_=lnv, func=AF.Exp, scale=-0.5)
            last = (rep == c.repeats - 1) and (l == L - 1)
            nc.vector.tensor_mul(xs, xa, _bc(rstd.unsqueeze(2),
                                             (128, c.SC, c.D)))

            if not last:
                # scatter-write shard feature-major, AllGather, reload xt
                lx = l if l < L - 1 else 0
                for sc in range(c.SC):
                    dst = bass.AP(
                        tensor=xpiece_d[lx].tensor, offset=sc * 128,
                        ap=[[1, 128], [c.TS, c.D]])
                    with nc.allow_non_contiguous_dma(reason="transpose"):
                        nc.sync.dma_start(out=dst, in_=xs[:, sc, :])
                nc.gpsimd.collective_compute(
                    "AllGather", OP.bypass, replica_groups=groups,
                    ins=[xpiece_d[lx]], outs=[xall_d[lx]])
                for r in range(c.group):
                    nc.sync.dma_start(
                        out=xt[:, :, r * c.TS:(r + 1) * c.TS],
                        in_=xall_d[lx][r * c.D:(r + 1) * c.D, :].rearrange(
                            "(f p) t -> p f t", p=128))
            else:
                # final layernorm on the shard -> output
                xf = xs
                mean2 = stats.tile([128, c.SC], F32, tag="mu2")
                nc.vector.tensor_reduce(out=mean2, in_=xf,
                                        axis=mybir.AxisListType.X, op=OP.add)
                nc.vector.tensor_scalar_mul(mean2, mean2, -1.0 / c.D)
                nc.vector.tensor_add(xf, xf, _bc(mean2.unsqueeze(2),
                                                 (128, c.SC, c.D)))
                sq2 = ar[:, :c.SC, :]
                nc.vector.tensor_mul(sq2, xf, xf)
                var2 = stats.tile([128, c.SC], F32, tag="var2")
                nc.vector.tensor_reduce(out=var2, in_=sq2,
                                        axis=mybir.AxisListType.X, op=OP.add)
                lnv2 = stats.tile([128, c.SC], F32, tag="lnv2")
                nc.scalar.activation(out=lnv2, in_=var2, func=AF.Ln,
                                     scale=1.0 / c.D, bias=eps_c)
                rstd2 = stats.tile([128, c.SC], F32, tag="rstd2")
                nc.scalar.activation(out=rstd2, in_=lnv2, func=AF.Exp,
                                     scale=-0.5)
                fo = ar[:, c.SC:2 * c.SC, :]
                nc.vector.tensor_mul(fo, xf, _bc(rstd2.unsqueeze(2),
                                                 (128, c.SC, c.D)))
                nc.sync.dma_start(
                    out=out_d.rearrange("(s p) d -> p s d", p=128),
                    in_=fo)

    nc.compile()
    return nc


# ---------------------------------------------------------------------------
# host side
# ---------------------------------------------------------------------------

def make_in_maps(cfg: Cfg, q, Wq, Wv, Wo, gammas):
    c = cfg
    q = np.asarray(q, np.float32)
    Wq = np.asarray(Wq, np.float32)
    Wv = np.asarray(Wv, np.float32)
    Wo = np.asarray(Wo, np.float32)
    gammas = np.asarray(gammas, np.float32)
    S = c.S

    p = np.arange(128)[:, None]
    # lnpos[p, c] = ln(|p + S - c|), read at c = k + S - st*128
    cc = np.arange(2 * S)[None, :]
    posv = np.abs(p + S - cc).astype(np.float32)
    with np.errstate(divide="ignore"):
        lnpos = np.where(posv > 0, np.log(posv), NEGBIG).astype(np.float32)
    # cmask[p, c'] = 0 if (c' - S) < p else NEGBIG, read at c' = k + S - st*128
    cp = np.arange(S + 128)[None, :]
    cmask = np.where((cp - S) < p, 0.0, NEGBIG).astype(np.float32)

    in_maps = []
    for core in range(c.n_cores):
        b, hg = divmod(core, c.group)
        h0 = hg * c.HC
        cols = slice(h0 * c.DH, (h0 + c.HC) * c.DH)
        # 2*ln|gamma| so that exp(0.5*(L - lnZ + ln g^2)) = |g|*dist
        gn = 2.0 * np.log(np.maximum(np.abs(gammas[:, h0:h0 + c.HC]), 1e-20))
        in_maps.append({
            "x0T": np.ascontiguousarray(q[b].T),
            "x0s": np.ascontiguousarray(q[b][hg * c.TS:(hg + 1) * c.TS]),
            "wqv": np.ascontiguousarray(
                np.concatenate([Wq[:, :, cols], Wv[:, :, cols]], axis=2)),
            "wo": np.ascontiguousarray(Wo[:, cols, :]),
            "gneg": np.broadcast_to(gn[None], (128, c.L, c.HC)).copy(),
            "lnpos": lnpos,
            "cmask": cmask,
        })
    return in_maps


def assemble_out(cfg: Cfg, results):
    c = cfg
    out = np.empty((c.B, c.S, c.D), np.float32)
    for core in range(c.n_cores):
        b, hg = divmod(core, c.group)
        out[b, hg * c.TS:(hg + 1) * c.TS] = results[core]["out"]
    return out


_PROGRAM_CACHE = {}


def get_program(cfg: Cfg):
    nc = _PROGRAM_CACHE.get(cfg.key)
    if nc is None:
        nc = build_program(cfg)
        _PROGRAM_CACHE[cfg.key] = nc
    return nc


def kernel(**inputs):
    cfg = Cfg()
    nc = get_program(cfg)
    in_maps = make_in_maps(
        cfg, inputs["q"], inputs["Wq"], inputs["Wv"], inputs["Wo"],
        inputs["gammas"])
    last_exc = None
    for _ in range(3):  # retry transient terminal hangs / device resets
        try:
            res = run_bass_kernel_spmd(nc, in_maps, list(range(cfg.n_cores)))
            return assemble_out(cfg, res.results)
        except Exception as exc:  # pragma: no cover - infra flake path
            last_exc = exc
    raise last_exc
